# revision 52
# baseline (speedup 1.0000x reference)
"""CharacterIsolationAttention Trainium2 kernel.

Sharding (hardcoded): 8 cores = 2 batches x 4 head-groups.
core c handles batch b = c//4 and heads [4g, 4g+4), g = c%4.

Per-core device program (SPMD, same NEFF, different inputs):
  - qkv projection for its 4 heads from pre-transposed x (d, n) layout,
    producing qT/kT in (hd, n) layout and v in (n, hd) layout.
  - RMS-norm of q/k applied in (hd, n) layout (sum-of-squares over the
    partition dim via a ones-vector matmul); chunk-local in n.
  - scores computed TRANSPOSED: S.T[k, q] via an augmented contraction
    [kT; cm; 1] x [qT*scale*rstd; 2*g3*cm*recip_rowmax; -g3] so the
    character-isolation bias rides along in the same matmul. The
    interaction-mask term is accumulated into the same PSUM tile with a
    (g3*I) identity matmul against host-prescaled bf16 0.3*im.T tiles.
  - P.T = exp(PSUM) via ScalarE, straight to bf16.
  - PV matmul with a ones column appended to v gives both out.T (rows
    0..63) and the softmax denominator (row 64); normalize, then the
    output projection produces this core's partial out.T (1024, 2048).
Host: prepares transposed/prescaled inputs and the tiny O(C*N^2) bias
row-max tails; sums the 4 head-group partials per batch at the end.
"""

import os
import sys

for _p in ("/root/.axon_site", "/root/.axon_site/_ro/trn_rl_repo", "/root/.axon_site/_ro/pypackages"):
    if os.path.isdir(_p) and _p not in sys.path:
        sys.path.append(_p)

import ml_dtypes
import numpy as np

import concourse.bass as bass
import concourse.tile as tile
from concourse import bacc, mybir
from concourse.bass_utils import run_bass_kernel_spmd

B, N, D = 2, 2048, 1024
H, HD, C = 16, 64, 4
NHG = 4          # heads per core
EPS = 1e-6
SCALE = HD ** -0.5
F32 = mybir.dt.float32
F32R = mybir.dt.float32r
BF16 = mybir.dt.bfloat16
AX = mybir.AxisListType
OP = mybir.AluOpType
ACTF = mybir.ActivationFunctionType

NT = N // 128    # 16 k-tiles
NQC = N // 512   # 4 q chunks of 512
ND = D // 128    # 8 contraction tiles


def _bcast_part(ap, nparts):
    """Partition-broadcast a (1, ...) DRAM AP to nparts partitions."""
    return bass.AP(tensor=ap.tensor, offset=ap.offset, ap=[[0, nparts]] + list(ap.ap[1:]))


def build_program():
    nc = bacc.Bacc("TRN2", target_bir_lowering=False, debug=False, num_devices=8)

    xT = nc.dram_tensor("xT", (D, N), F32R, kind="ExternalInput").ap()
    imS = nc.dram_tensor("imS", (N, N), BF16, kind="ExternalInput").ap()   # 0.3*im[b].T, bf16
    wqkT = nc.dram_tensor("wqkT", (D, 2 * NHG * HD), F32R, kind="ExternalInput").ap()
    wvT = nc.dram_tensor("wvT", (D, NHG * HD), F32R, kind="ExternalInput").ap()
    outwT = nc.dram_tensor("outwT", (NHG * HD, D), BF16, kind="ExternalInput").ap()
    qtail = nc.dram_tensor("qtail", (NHG, 5, N), F32R, kind="ExternalInput").ap()
    ktail = nc.dram_tensor("ktail", (5, N), F32R, kind="ExternalInput").ap()
    g3d = nc.dram_tensor("g3d", (128, NHG, 128), BF16, kind="ExternalInput").ap()
    g3c = nc.dram_tensor("g3c", (128, NHG), F32, kind="ExternalInput").ap()
    qkw = nc.dram_tensor("qkw", (128, 2), F32, kind="ExternalInput").ap()  # [:,0]=qw [:,1]=kw, dup both halves
    blkA = nc.dram_tensor("blkA", (128, 2), F32R, kind="ExternalInput").ap()  # col h = half-h indicator
    blkB = nc.dram_tensor("blkB", (2, 128), F32R, kind="ExternalInput").ap()
    outT = nc.dram_tensor("outT", (D, N), F32, kind="ExternalOutput").ap()

    with tile.TileContext(nc) as tc:
        with (
            tc.tile_pool(name="persist", bufs=1) as pp,
            tc.tile_pool(name="scratch", bufs=2) as sp,
            tc.tile_pool(name="work512", bufs=3, space="PSUM") as ps512,
            tc.tile_pool(name="dramsc", bufs=3, space="DRAM") as dp,
        ):
            def bcast_via_dram(row_ap, nparts, dst):
                """row_ap (1, n) SBUF -> dst (nparts, n) SBUF via DRAM scratch."""
                n = row_ap.shape[-1]
                dsc = dp.tile([1, n], F32, name="dsc", tag=f"dsc{n}")
                nc.sync.dma_start(out=dsc, in_=row_ap)
                nc.sync.dma_start(out=dst, in_=_bcast_part(dsc, nparts))

            # ---------- constants / small inputs ----------
            qkw_sb = pp.tile([128, 2], F32, name="qkw_sb")
            nc.gpsimd.dma_start(out=qkw_sb, in_=qkw)
            blkA_sb = pp.tile([128, 2], F32R, name="blkA_sb")
            nc.gpsimd.dma_start(out=blkA_sb, in_=blkA)
            blkB_sb = pp.tile([2, 128], F32R, name="blkB_sb")
            nc.gpsimd.dma_start(out=blkB_sb, in_=blkB)
            g3diag = pp.tile([128, NHG, 128], BF16, name="g3diag")
            nc.gpsimd.dma_start(out=g3diag, in_=g3d)
            g3col = pp.tile([128, NHG], F32, name="g3col")
            nc.gpsimd.dma_start(out=g3col, in_=g3c)

            eps2 = pp.tile([2, 2], F32, name="eps2")
            nc.vector.memset(eps2[:, 0:1], EPS)         # k side: sqrt(ss/64 + eps)
            nc.vector.memset(eps2[:, 1:2], EPS * HD)    # q side: sqrt(ss + 64*eps) = 8*sqrt(.)
            # ones row (1, 64) f32r for the rstd partition-broadcast matmul
            ones_r_f = pp.tile([1, HD], F32, name="ones_r_f")
            ones_row64 = pp.tile([1, HD], F32R, name="ones_row64")
            nc.vector.memset(ones_r_f, 1.0)
            nc.vector.tensor_copy(out=ones_row64, in_=ones_r_f)

            # augmented q/k buffers: rows 0..63 normed qT/kT, 64..68 host tails
            qaug = [pp.tile([69, N], F32R, name=f"qaug{h}") for h in range(NHG)]
            kaug = [pp.tile([69, N], F32R, name=f"kaug{h}") for h in range(NHG)]
            for h in range(NHG):
                nc.gpsimd.dma_start(out=qaug[h][64:69, :], in_=qtail[h])
                nc.gpsimd.dma_start(out=kaug[h][64:69, :], in_=ktail)

            # ---------- P1: projections (x streamed in 512-col chunks) ----------
            # per-dt weight loads so the first proj matmul starts after 256KB
            wqk_sb = pp.tile([128, ND, 2 * NHG * HD], F32R, name="wqk_sb")
            wv_sb = pp.tile([128, ND, NHG * HD], F32R, name="wv_sb")
            for dt_i in range(ND):
                nc.sync.dma_start(
                    out=wqk_sb[:, dt_i, :],
                    in_=bass.AP(tensor=wqkT.tensor, offset=wqkT.offset + dt_i * 128 * 512,
                                ap=[[512, 128], [1, 512]]))
                nc.sync.dma_start(
                    out=wv_sb[:, dt_i, :],
                    in_=bass.AP(tensor=wvT.tensor, offset=wvT.offset + dt_i * 128 * 256,
                                ap=[[256, 128], [1, 256]]))

            vq = pp.tile([128, NT, NHG, HD + 1], BF16, name="vq")
            nc.vector.memset(vq[:, :, :, HD:HD + 1], 1.0)

            with (
                tc.tile_pool(name="xpool", bufs=2) as xp,
                tc.tile_pool(name="pv", bufs=2, space="PSUM") as psv,
                tc.tile_pool(name="ptiny2", bufs=1, space="PSUM") as pst2,
            ):
                for qc in range(NQC):
                    xc = []
                    for dt_i in range(ND):
                        xt = xp.tile([128, 512], F32R, name=f"x{dt_i}", tag=f"x{dt_i}")
                        nc.sync.dma_start(out=xt, in_=xT[dt_i * 128:(dt_i + 1) * 128,
                                                         qc * 512:(qc + 1) * 512])
                        xc.append(xt)
                    # qk projection: et 0,1 -> q heads ; et 2,3 -> k heads
                    for et in range(4):
                        ps = ps512.tile([128, 512], F32, name="ps", tag="s512")
                        for dt_i in range(ND):
                            nc.tensor.matmul(
                                ps,
                                lhsT=wqk_sb[:, dt_i, et * 128:(et + 1) * 128],
                                rhs=xc[dt_i],
                                start=(dt_i == 0), stop=(dt_i == ND - 1))
                        is_q = et < 2
                        qkraw = sp.tile([128, 512], F32, name="qkraw", tag="qkraw", bufs=8)
                        nc.scalar.copy(out=qkraw, in_=ps)
                        sq = sp.tile([128, 512], F32R, name="sq", tag="sq", bufs=2)
                        nc.gpsimd.tensor_mul(sq, qkraw, qkraw)
                        ssp = pst2.tile([2, 512], F32, name="ssp", tag="ss")
                        nc.tensor.matmul(ssp, lhsT=blkA_sb, rhs=sq, start=True, stop=True)
                        rstd = sp.tile([2, 512], F32R, name="rstd", tag="rstd", bufs=2)
                        # q side folds the 1/8 attn scale: 1/sqrt(ss + 64*eps)
                        nc.scalar.activation(out=rstd, in_=ssp, func=ACTF.Sqrt,
                                             bias=eps2[:, 1:2] if is_q else eps2[:, 0:1],
                                             scale=1.0 if is_q else 1.0 / HD)
                        with nc.allow_low_precision(reason="f32r rstd feeds bcast matmul"):
                            nc.vector.reciprocal(rstd, rstd)
                        rbp = pst2.tile([128, 512], F32, name="rbp", tag="rbp", bufs=1)
                        nc.tensor.matmul(rbp, lhsT=blkB_sb, rhs=rstd, start=True, stop=True)
                        rb = sp.tile([128, 512], F32, name="rb", tag="rb", bufs=3)
                        nc.scalar.copy(out=rb, in_=rbp)
                        for half in range(2):
                            h = (et % 2) * 2 + half
                            pslc = slice(half * 64, (half + 1) * 64)
                            dst = (qaug if is_q else kaug)[h][0:64, qc * 512:(qc + 1) * 512]
                            nc.vector.scalar_tensor_tensor(
                                out=dst, in0=qkraw[pslc, :],
                                scalar=qkw_sb[pslc, 0:1] if is_q else qkw_sb[pslc, 1:2],
                                in1=rb[pslc, :], op0=OP.mult, op1=OP.mult)
                    # v projection for this chunk's 4 n-tiles
                    for j in range(4):
                        nt_i = qc * 4 + j
                        vp = psv.tile([128, NHG * HD], F32, name="vp", tag="v")
                        for dt_i in range(ND):
                            nc.tensor.matmul(
                                vp,
                                lhsT=xc[dt_i][:, j * 128:(j + 1) * 128],
                                rhs=wv_sb[:, dt_i, :],
                                start=(dt_i == 0), stop=(dt_i == ND - 1))
                        nc.scalar.copy(out=vq[:, nt_i, :, 0:HD],
                                       in_=vp.rearrange("p (h d) -> p h d", h=NHG))

            # ---------- P2: attention ----------
            houT = [pp.tile([128, N], BF16, name=f"houT{t}") for t in range(2)]
            outw_sb = pp.tile([128, 2, D], BF16, name="outw_sb")
            nc.sync.dma_start(out=outw_sb,
                              in_=bass.AP(tensor=outwT.tensor, offset=outwT.offset,
                                          ap=[[D, 128], [128 * D, 2], [1, D]]))

            def out_proj_cols(qc_list):
                for et in range(ND):
                    for qc in qc_list:
                        ops3 = ps512.tile([128, 512], F32, name="ops3", tag="ops3", bufs=1)
                        for ct in range(2):
                            nc.tensor.matmul(
                                ops3,
                                lhsT=outw_sb[:, ct, et * 128:(et + 1) * 128],
                                rhs=houT[ct][:, qc * 512:(qc + 1) * 512],
                                start=(ct == 0), stop=(ct == 1))
                        ot = sp.tile([128, 512], F32, name="ot", tag="ot", bufs=4)
                        nc.scalar.copy(out=ot, in_=ops3)
                        nc.sync.dma_start(out=outT[et * 128:(et + 1) * 128,
                                                   qc * 512:(qc + 1) * 512], in_=ot)

            with (
                tc.tile_pool(name="impool", bufs=8) as imp,
                tc.tile_pool(name="ppool", bufs=8) as ppl,
                tc.tile_pool(name="outps_pool", bufs=2, space="PSUM") as pso,
            ):
                for qhf in range(2):
                    for pair in range(2):
                        q0 = qhf * 1024
                        ops_ = [pso.tile([HD + 1, 1024], F32, name=f"o{hh}", tag=f"o{hh}",
                                         bufs=1)
                                for hh in range(2)]
                        for kt in range(NT):
                            imt = imp.tile([128, 1024], BF16, name="imt", tag="imt")
                            nc.sync.dma_start(
                                out=imt, in_=imS[kt * 128:(kt + 1) * 128, q0:q0 + 1024])
                            for hh in range(2):
                                h = pair * 2 + hh
                                for qc in range(2):
                                    sps = ps512.tile([128, 512], F32, name="sps", tag="s512")
                                    nc.tensor.matmul(
                                        sps,
                                        lhsT=kaug[h][:, kt * 128:(kt + 1) * 128],
                                        rhs=qaug[h][:, q0 + qc * 512:q0 + (qc + 1) * 512],
                                        start=True, stop=False)
                                    nc.tensor.matmul(
                                        sps, lhsT=g3diag[:, h, :],
                                        rhs=imt[:, qc * 512:(qc + 1) * 512],
                                        start=False, stop=True)
                                    pch = ppl.tile([128, 512], BF16, name="pch", tag="pch")
                                    nc.scalar.activation(out=pch, in_=sps, func=ACTF.Exp)
                                    nc.tensor.matmul(
                                        ops_[hh][:, qc * 512:(qc + 1) * 512],
                                        lhsT=vq[:, kt, h, :], rhs=pch,
                                        start=(kt == 0), stop=(kt == NT - 1),
                                        skip_group_check=True)
                        for hh in range(2):
                            rd = sp.tile([1, 1024], F32R, name="rd", tag="rd", bufs=2)
                            with nc.allow_low_precision(reason="f32r recip feeds bcast matmul"):
                                nc.vector.reciprocal(rd, ops_[hh][HD:HD + 1, :])
                            rb64 = sp.tile([64, 1024], F32, name="rb64", tag="rb64", bufs=2)
                            for dc in range(2):
                                rb64p = ps512.tile([64, 512], F32, name="rb64p", tag="ops3", bufs=1)
                                nc.tensor.matmul(rb64p, lhsT=ones_row64,
                                                 rhs=rd[:, dc * 512:(dc + 1) * 512],
                                                 start=True, stop=True)
                                nc.vector.tensor_copy(out=rb64[:, dc * 512:(dc + 1) * 512],
                                                      in_=rb64p)
                            nc.vector.tensor_mul(
                                houT[pair][hh * 64:(hh + 1) * 64, q0:q0 + 1024],
                                ops_[hh][0:HD, :], rb64)
                    # output projection for this column half once both pairs done
                    if pair == 1:
                        out_proj_cols([qhf * 2, qhf * 2 + 1])


    nc.compile()
    return nc


_NC_CACHE = {}


def _get_program():
    if "nc" not in _NC_CACHE:
        _NC_CACHE["nc"] = build_program()
    return _NC_CACHE["nc"]


def _make_in_maps(inputs):
    x = np.asarray(inputs["x"], np.float32)
    character_masks = np.asarray(inputs["character_masks"], np.float32)
    interaction_mask = np.asarray(inputs["interaction_mask"], np.float32)
    qkv_w = np.asarray(inputs["qkv_w"], np.float32)
    out_w = np.asarray(inputs["out_w"], np.float32)
    q_norm_w = np.asarray(inputs["q_norm_w"], np.float32).reshape(HD, 1)
    k_norm_w = np.asarray(inputs["k_norm_w"], np.float32).reshape(HD, 1)
    isolation_gate = np.asarray(inputs["isolation_gate"], np.float32)
    qkw_h = np.ascontiguousarray(
        np.tile(np.concatenate([q_norm_w, k_norm_w], axis=1), (2, 1)))  # (128, 2)
    blkA_h = np.zeros((128, 2), np.float32)
    blkA_h[0:64, 0] = 1.0
    blkA_h[64:128, 1] = 1.0
    blkB_h = np.ascontiguousarray(blkA_h.T)
    g3_full = 3.0 * np.clip(isolation_gate, 0.0, 1.0)         # (H,)

    xT_b = [np.ascontiguousarray(x[b].T) for b in range(B)]
    imS_b = [np.ascontiguousarray((0.3 * interaction_mask[b].T).astype(ml_dtypes.bfloat16))
             for b in range(B)]
    # host bias prep: row-max of same_char (tiny O(C*N^2)), per batch
    cmrec_b = []
    ktail_b = []
    for b in range(B):
        cmb = character_masks[b]                               # (C, N)
        sc = cmb.T @ cmb                                       # (N, N)
        m = np.maximum(sc.max(axis=-1), 1e-6)                  # (N,)
        cmrec_b.append(cmb / m[None, :])
        ktail_b.append(np.ascontiguousarray(
            np.concatenate([cmb, np.ones((1, N), np.float32)], axis=0)))

    in_maps = []
    for core in range(8):
        b, g = core // 4, core % 4
        cs = slice(g * NHG * HD, (g + 1) * NHG * HD)   # 256-wide head-group slice
        g3 = g3_full[g * NHG:(g + 1) * NHG]
        wq = qkv_w[cs, :]                              # (256, D)
        wk = qkv_w[D:2 * D, :][cs, :]
        wv = qkv_w[2 * D:3 * D, :][cs, :]
        wqkT_c = np.ascontiguousarray(np.concatenate([wq, wk], axis=0).T)  # (D, 512)
        wvT_c = np.ascontiguousarray(wv.T)                                  # (D, 256)
        outwT_c = np.ascontiguousarray(out_w[:, cs].T.astype(ml_dtypes.bfloat16))  # (256, D)
        qtail_c = np.empty((NHG, 5, N), np.float32)
        for hh in range(NHG):
            qtail_c[hh, 0:C] = 2.0 * g3[hh] * cmrec_b[b]
            qtail_c[hh, C] = -g3[hh]
        g3d_c = np.zeros((128, NHG, 128), ml_dtypes.bfloat16)
        idx = np.arange(128)
        for hh in range(NHG):
            g3d_c[idx, hh, idx] = g3[hh].astype(ml_dtypes.bfloat16)
        in_maps.append({
            "xT": xT_b[b],
            "imS": imS_b[b],
            "wqkT": wqkT_c,
            "wvT": wvT_c,
            "outwT": outwT_c,
            "qtail": qtail_c,
            "ktail": ktail_b[b],
            "g3d": g3d_c,
            "g3c": np.ascontiguousarray(np.tile(g3.astype(np.float32), (128, 1))),
            "qkw": qkw_h,
            "blkA": blkA_h,
            "blkB": blkB_h,
        })
    return in_maps


def run(inputs, trace=False, **kw):
    nc = _get_program()
    in_maps = _make_in_maps(inputs)
    res = run_bass_kernel_spmd(nc, in_maps, core_ids=list(range(8)), trace=trace, **kw)
    out = np.zeros((B, N, D), np.float32)
    for core in range(8):
        b = core // 4
        out[b] += res.results[core]["outT"].T
    return out, res


def kernel(**inputs):
    out, _ = run(inputs, trace=False)
    return out
